# revision 19
# baseline (speedup 1.0000x reference)
"""Based-style linear attention (Taylor feature map) on 8 Trainium2 cores.

Math: per head h (FDIM=16, HEAD_DIM=64):
    q,k = HS@Wq, HS@Wk (16 dims/head), v = HS@Wv (64 dims/head)
    phi(q).phi(k) = Square(S/sqrt(32) + 1/sqrt(2)) + 1/2,  S = q.k
    y_t = sum_{s<=t} sq'_st v_s / sum_{s<=t} sq'_st ;  out = concat_h(y) @ Wo

Sharding: head-parallel, 2 virtual heads per core (12 real + 4 zero),
no collectives; host sums the 8 partial outputs.

v2 design vs baseline:
 - single-pass packed q/k projection: stationary [128,128] produces k at
   PSUM rows 0-63 and q at rows 64-127 in one matmul stream; the q half
   lands at SBUF base 0 via a partition-shifting DVE copy.
 - affine fold: k scaled by 1/sqrt(32) on host, constant rows (b on the
   k side, 1 on the q side) via memset, so PSUM holds S' = aS + b and the
   Square needs no scale/bias.
 - squares split between ACT (plain Square) and DVE (tensor_mul);
   diagonal-block causal mask fused with the +1/2 intra term on GpSimd:
   (sq + 0.5) * tri in one scalar_tensor_tensor.
 - v projection accumulated kb-outer (overlaps the hs DMA stream) into
   one PSUM tile [128, 8, 128]; per-head score matmuls run on PE row
   groups 0/32 concurrently (data placed at partitions 0-31/32-63).
 - chunk epilogues (sel +1/2 term at row groups 0/32, reciprocal, den
   broadcast via GpSimd partition_broadcast, y = num/den, o-proj at row
   groups 0/64, output copy + DMA) pipelined in 3 batches after passes
   3/5/7 so PE never idles long (HAM clock gate stays at 2.4 GHz).
 - PE warm-up: dummy matmuls during the initial DMA wait.
 - host-side layouts pre-scrambled so every DMA is 128 contiguous
   per-partition chunks.
"""

import math

import numpy as np
import ml_dtypes

import concourse.bass as bass
import concourse.mybir as mybir
import concourse.tile as tile
from concourse import bacc
from concourse.bass_utils import run_bass_kernel_spmd

L = 1024
D = 768
H = 12
FD = 16
HD = 64
NCORE = 8
NCH = 8
KB = 6
F32 = mybir.dt.float32
BF16 = mybir.dt.bfloat16

A_SCALE = 1.0 / math.sqrt(32.0)
A_BIAS = 1.0 / math.sqrt(2.0)

# feature flags gated on hardware probes
SINGLE_PASS_PROJ = True  # partition-shift DVE copy (probe T6: PASS)
WARM_MMS = 14

EPI_BATCHES = {3: (0, 4), 5: (4, 6), 7: (6, 8)}

_compiled_nc = None
_last_in_maps = None
_last_res = None


def _bank_splits(lo, hi, bank=512):
    out = []
    a = lo
    while a < hi:
        b = min(hi, (a // bank + 1) * bank)
        out.append((a, b))
        a = b
    return out


def _use_act(j, blk, h):
    """Square-engine assignment: alternate ACT/DVE early; ACT-only in the
    late passes where DVE runs the epilogues."""
    if j >= 4:
        return True
    return (blk + h) % 2 == 0


def _build_nc():
    nc = bacc.Bacc("TRN2", target_bir_lowering=False, debug=False, num_devices=NCORE)

    hsp = nc.dram_tensor("hsp", [128, KB, L], BF16, kind="ExternalInput")
    wproj = nc.dram_tensor("wproj", [128, KB, 256], BF16, kind="ExternalInput")
    wod = nc.dram_tensor("wod", [128, D], BF16, kind="ExternalInput")
    cst = nc.dram_tensor("cst", [128, 192], BF16, kind="ExternalInput")
    seld = nc.dram_tensor("seld", [8, L], BF16, kind="ExternalInput")
    outd = nc.dram_tensor("out", [128, NCH * D], BF16, kind="ExternalOutput")

    MUL = mybir.AluOpType.mult
    ADD = mybir.AluOpType.add
    SQF = mybir.ActivationFunctionType.Square

    with tile.TileContext(nc) as tc:
        with (
            tc.tile_pool(name="cst", bufs=1) as cp,
            tc.tile_pool(name="sqp", bufs=6) as sqp,
            tc.tile_pool(name="wrk", bufs=2) as wrk,
        ):
            hs_sb = [cp.tile([128, L], BF16, tag=f"hs{kb}", name=f"hs{kb}") for kb in range(KB)]
            wp_sb = cp.tile([128, KB, 256], BF16, tag="wp")
            wo_sb = cp.tile([128, D], BF16, tag="wo")
            cst_sb = cp.tile([128, 192], BF16, tag="cstt")
            sel_sb = cp.tile([8, L], BF16, tag="sel")
            kq_sb = cp.tile([64, 2048], BF16, tag="kq")
            vx_sb = cp.tile([128, NCH, 130], BF16, tag="vx")
            colsum_sb = cp.tile([8, 130], BF16, tag="colsum")
            osb = cp.tile([128, NCH, D], BF16, tag="osb")
            warm_sb = cp.tile([128, 128], BF16, tag="warm")
            ones64_sb = cp.tile([65, 64], F32, tag="ones64")
            wact_sb = cp.tile([128, 2], BF16, tag="wact")
            tri_sb = cst_sb[:, 0:128]
            ones8_sb = cst_sb[:, 128:192]
            yT2 = cp.tile([128, L], BF16, tag="yT2")
            yT = [yT2[0:64, :], yT2[64:128, :]]

            # ---- input DMAs, priority-ordered ----
            nc.sync.dma_start(out=wp_sb, in_=wproj.ap())
            for kb in range(4):
                nc.sync.dma_start(out=hs_sb[kb], in_=hsp.ap()[:, kb, :])
            nc.sync.dma_start(out=cst_sb, in_=cst.ap())
            nc.sync.dma_start(out=sel_sb, in_=seld.ap())
            for kb in (4, 5):
                nc.sync.dma_start(out=hs_sb[kb], in_=hsp.ap()[:, kb, :])
            nc.sync.dma_start(out=wo_sb, in_=wod.ap())

            # ---- PE warm-up + ACT table preload during the DMA wait ----
            nc.gpsimd.memset(warm_sb, 0.0)
            nc.vector.memset(ones64_sb, 0.0)
            nc.vector.memset(ones64_sb[64:65, :], 1.0)
            nc.scalar.activation(out=wact_sb, in_=warm_sb[:, 0:2], func=SQF)
            with tc.tile_pool(name="pwarm", bufs=1, space="PSUM") as pw:
                pwt = pw.tile([128, 128], F32, tag="pw")
                for _ in range(WARM_MMS):
                    nc.tensor.matmul(pwt, warm_sb, warm_sb, start=True, stop=True)

            # ---- projections ----
            with (
                tc.tile_pool(name="pproj", bufs=1, space="PSUM") as pp,
                tc.tile_pool(name="pvp", bufs=3, space="PSUM") as pvp,
            ):
                if SINGLE_PASS_PROJ:
                    pkq = pp.tile([128, L], F32, tag="pkq")
                    for kb in range(KB):
                        for a, b in ((0, 512), (512, 1024)):
                            nc.tensor.matmul(
                                pkq[:, a:b], wp_sb[:, kb, 0:128], hs_sb[kb][:, a:b],
                                start=(kb == 0), stop=(kb == KB - 1),
                            )
                    nc.vector.tensor_copy(kq_sb[:, 0:1024], pkq[0:64, :])
                    # partition-shifting copy: q half (PSUM rows 64-127) -> base 0
                    nc.vector.tensor_copy(kq_sb[:, 1024:2048], pkq[64:128, :])
                else:
                    pk = pp.tile([64, L], F32, tag="pk")
                    pq = pp.tile([64, L], F32, tag="pq")
                    for kb in range(KB):
                        for a, b in ((0, 512), (512, 1024)):
                            nc.tensor.matmul(
                                pk[:, a:b], wp_sb[:, kb, 0:64], hs_sb[kb][:, a:b],
                                start=(kb == 0), stop=(kb == KB - 1),
                            )
                            nc.tensor.matmul(
                                pq[:, a:b], wp_sb[:, kb, 64:128], hs_sb[kb][:, a:b],
                                start=(kb == 0), stop=(kb == KB - 1),
                            )
                    nc.vector.tensor_copy(kq_sb[:, 0:1024], pk)
                    nc.vector.tensor_copy(kq_sb[:, 1024:2048], pq)

                # v projection: one full-bank PSUM tile per chunk so each
                # bank holds exactly one accumulation group (start=True
                # clears at bank granularity)
                for ch in range(NCH):
                    pv = pvp.tile([128, 512], F32, tag="pv", name=f"pv{ch}")
                    for kb in range(KB):
                        nc.tensor.matmul(
                            pv[:, 0:128],
                            hs_sb[kb][:, ch * 128 : (ch + 1) * 128],
                            wp_sb[:, kb, 128:256],
                            start=(kb == 0), stop=(kb == KB - 1),
                        )
                    nc.vector.tensor_copy(vx_sb[:, ch, 0:64], pv[:, 0:64])
                    nc.vector.tensor_copy(vx_sb[:, ch, 65:129], pv[:, 64:128])

                # constant rows: b on the k side, 1 on the q side
                for r in (0, 32):
                    nc.vector.memset(kq_sb[r : r + 1, 0:1024], A_BIAS)
                    nc.vector.memset(kq_sb[r : r + 1, 1024:2048], 1.0)

                nc.gpsimd.memset(vx_sb[:, :, 64], 1.0)
                nc.gpsimd.memset(vx_sb[:, :, 129], 1.0)

            # per-chunk column sums of vx (inter-chunk +1/2 term);
            # h1 placed at partitions 32-39 for row-group concurrency
            with tc.tile_pool(name="pcsp", bufs=1, space="PSUM") as pcp:
                pcs = pcp.tile([8, 130], F32, tag="pcs")
                for ch in range(NCH):
                    nc.tensor.matmul(
                        pcs, ones8_sb[:, ch * 8 : (ch + 1) * 8], vx_sb[:, ch, :],
                        start=(ch == 0), stop=(ch == NCH - 1),
                    )
                nc.vector.tensor_copy(colsum_sb, pcs)

            # ================= attention =================
            with (
                tc.tile_pool(name="pnum", bufs=1, space="PSUM") as pn,
                tc.tile_pool(name="ppa", bufs=2, space="PSUM") as ppa,
                tc.tile_pool(name="ppo", bufs=1, space="PSUM") as ppo,
                tc.tile_pool(name="prbp", bufs=1, space="PSUM") as prbp,
            ):
                nums = [pn.tile([65, L], F32, tag=f"num{h}", name=f"num{h}") for h in range(2)]
                for j in range(NCH):
                    tlo = j * 128
                    w = L - tlo
                    nblk = (w + 511) // 512
                    sqs = []
                    for blk in range(nblk):
                        bw = min(512, w - blk * 512)
                        for h in range(2):
                            pa = ppa.tile([128, 512], F32, tag="pa", name=f"pa{j}_{blk}_{h}")[:, :bw]
                            nc.tensor.matmul(
                                pa,
                                kq_sb[32 * h : 32 * h + 32, tlo : tlo + 128],
                                kq_sb[
                                    32 * h : 32 * h + 32,
                                    1024 + tlo + blk * 512 : 1024 + tlo + blk * 512 + bw,
                                ],
                                start=True, stop=True,
                            )
                            sq = sqp.tile([128, 512], BF16, tag="sq", name=f"sq{j}_{blk}_{h}")[:, :bw]
                            if _use_act(j, blk, h):
                                nc.scalar.activation(out=sq, in_=pa, func=SQF)
                            else:
                                # DVE can't read two PSUM operands: bounce to
                                # SBUF (2x-accel copy), square on GpSimd
                                pb = sqp.tile([128, 512], BF16, tag="pb", name=f"pb{j}_{blk}_{h}")[:, :bw]
                                nc.vector.tensor_copy(pb, pa)
                                nc.gpsimd.tensor_mul(sq, pb, pb)
                            if blk == 0:
                                # causal mask + intra-chunk +1/2: (sq+0.5)*tri
                                nc.vector.scalar_tensor_tensor(
                                    out=sq[:, 0:128], in0=sq[:, 0:128],
                                    scalar=0.5, in1=tri_sb, op0=ADD, op1=MUL,
                                )
                            sqs.append((blk, h, sq, bw))
                    for blk, h, sq, bw in sqs:
                        lo = tlo + blk * 512
                        for a, b in _bank_splits(lo, lo + bw):
                            nc.tensor.matmul(
                                nums[h][:, a:b],
                                vx_sb[:, j, 65 * h : 65 * h + 65],
                                sq[:, a - lo : b - lo],
                                start=(j == 0), stop=False,
                            )

                    if j in EPI_BATCHES:
                        c0, c1 = EPI_BATCHES[j]
                        # inter-chunk +1/2 term, closes each chunk's accumulation
                        for i in range(c0, c1):
                            for h in range(2):
                                nc.tensor.matmul(
                                    nums[h][:, i * 128 : (i + 1) * 128],
                                    colsum_sb[:, 65 * h : 65 * h + 65],
                                    sel_sb[:, i * 128 : (i + 1) * 128],
                                    start=False, stop=True,
                                )
                        lo, hi = c0 * 128, c1 * 128
                        ncols = hi - lo
                        for h in range(2):
                            rc = wrk.tile([65, 512], F32, tag="rc", name=f"rc{j}_{h}")[:, :ncols]
                            nc.vector.reciprocal_approx_fast(out=rc, in_=nums[h][:, lo:hi])
                            rb = wrk.tile([64, 512], F32, tag="rb", name=f"rb{j}_{h}")[:, :ncols]
                            for p0 in range(0, ncols, 256):
                                p1 = min(ncols, p0 + 256)
                                prb = prbp.tile([64, 256], F32, tag="prb", name=f"prb{j}_{h}_{p0}")[:, : p1 - p0]
                                nc.tensor.matmul(
                                    prb, ones64_sb[64:65, :], rc[64:65, p0:p1],
                                    start=True, stop=True,
                                )
                                nc.vector.tensor_copy(rb[:, p0:p1], prb)
                            # h1 write is partition-shifting (probe T9)
                            nc.vector.tensor_mul(yT[h][:, lo:hi], nums[h][0:64, lo:hi], rb)
                        for i in range(c0, c1):
                            for a, b in ((0, 512), (512, D)):
                                po = ppo.tile([128, 512], F32, tag="po", name=f"po{i}_{a}")[:, : b - a]
                                nc.tensor.matmul(
                                    po, yT2[:, i * 128 : (i + 1) * 128],
                                    wo_sb[:, a:b],
                                    start=True, stop=True,
                                )
                                if i % 2 == 0:
                                    nc.scalar.copy(out=osb[:, i, a:b], in_=po)
                                else:
                                    nc.vector.tensor_copy(osb[:, i, a:b], po)
                            nc.sync.dma_start(
                                out=outd.ap()[:, i * D : (i + 1) * D], in_=osb[:, i, :]
                            )

    nc.finalize()
    return nc


def _host_consts():
    s = np.arange(128)[:, None]
    t = np.arange(128)[None, :]
    tri = (s <= t).astype(np.float32)
    ones8 = np.zeros((128, 64), dtype=np.float32)
    for ch in range(NCH):
        ones8[:, ch * 8 + ch] = 1.0
    sel = np.zeros((8, L), dtype=np.float32)
    for i in range(NCH):
        sel[:i, i * 128 : (i + 1) * 128] = 0.5
    return tri, ones8, sel


def kernel(hidden_states, Wq, Wk, Wv, Wo):
    global _compiled_nc, _last_in_maps
    hs = np.asarray(hidden_states, dtype=np.float32)[0]  # [L, D]
    Wq = np.asarray(Wq, dtype=np.float32)
    Wk = np.asarray(Wk, dtype=np.float32)
    Wv = np.asarray(Wv, dtype=np.float32)
    Wo = np.asarray(Wo, dtype=np.float32)

    if _compiled_nc is None:
        _compiled_nc = _build_nc()
    nc = _compiled_nc

    bf = ml_dtypes.bfloat16
    # hs scrambled: hsp[p, kb, l] = hs[l, kb*128+p]
    hsp = np.ascontiguousarray(
        hs.T.reshape(KB, 128, L).transpose(1, 0, 2)
    ).astype(bf)

    tri, ones8, sel = _host_consts()
    cstm = np.zeros((128, 192), dtype=np.float32)
    cstm[:, 0:128] = tri
    cstm[:, 128:192] = ones8
    cstm = cstm.astype(bf)
    sel = sel.astype(bf)

    in_maps = []
    for c in range(NCORE):
        heads = [2 * c, 2 * c + 1]
        wp = np.zeros((D, 256), dtype=np.float32)
        wo_c = np.zeros((128, D), dtype=np.float32)
        for hi, h in enumerate(heads):
            if h >= H:
                continue
            wp[:, 1 + 32 * hi : 1 + 32 * hi + FD] = A_SCALE * Wk[:, h * FD : (h + 1) * FD]
            wp[:, 65 + 32 * hi : 65 + 32 * hi + FD] = Wq[:, h * FD : (h + 1) * FD]
            wp[:, 128 + 64 * hi : 128 + 64 * hi + HD] = Wv[:, h * HD : (h + 1) * HD]
            wo_c[64 * hi : 64 * hi + HD, :] = Wo[h * HD : (h + 1) * HD, :]
        wp_s = np.ascontiguousarray(
            wp.reshape(KB, 128, 256).transpose(1, 0, 2)
        ).astype(bf)
        in_maps.append(
            {
                "hsp": hsp,
                "wproj": wp_s,
                "wod": wo_c.astype(bf),
                "cst": cstm,
                "seld": sel,
            }
        )

    _last_in_maps = in_maps
    res = run_bass_kernel_spmd(nc, in_maps, list(range(NCORE)))
    global _last_res
    _last_res = res
    acc = np.zeros((128, NCH, D), dtype=np.float32)
    for c in range(NCORE):
        acc += np.asarray(res.results[c]["out"], dtype=np.float32).reshape(
            128, NCH, D
        )
    out = acc.transpose(1, 0, 2).reshape(L, D)
    return out.reshape(1, L, D)


# revision 20
# speedup vs baseline: 1.0048x; 1.0048x over previous
"""Based-style linear attention (Taylor feature map) on 8 Trainium2 cores.

Math: per head h (FDIM=16, HEAD_DIM=64):
    q,k = HS@Wq, HS@Wk (16 dims/head), v = HS@Wv (64 dims/head)
    phi(q).phi(k) = Square(S/sqrt(32) + 1/sqrt(2)) + 1/2,  S = q.k
    y_t = sum_{s<=t} sq'_st v_s / sum_{s<=t} sq'_st ;  out = concat_h(y) @ Wo

Sharding: head-parallel, 2 virtual heads per core (12 real + 4 zero),
no collectives; host sums the 8 partial outputs.

v2 design vs baseline:
 - single-pass packed q/k projection: stationary [128,128] produces k at
   PSUM rows 0-63 and q at rows 64-127 in one matmul stream; the q half
   lands at SBUF base 0 via a partition-shifting DVE copy.
 - affine fold: k scaled by 1/sqrt(32) on host, constant rows (b on the
   k side, 1 on the q side) via memset, so PSUM holds S' = aS + b and the
   Square needs no scale/bias.
 - squares split between ACT (plain Square) and DVE (tensor_mul);
   diagonal-block causal mask fused with the +1/2 intra term on GpSimd:
   (sq + 0.5) * tri in one scalar_tensor_tensor.
 - v projection accumulated kb-outer (overlaps the hs DMA stream) into
   one PSUM tile [128, 8, 128]; per-head score matmuls run on PE row
   groups 0/32 concurrently (data placed at partitions 0-31/32-63).
 - chunk epilogues (sel +1/2 term at row groups 0/32, reciprocal, den
   broadcast via GpSimd partition_broadcast, y = num/den, o-proj at row
   groups 0/64, output copy + DMA) pipelined in 3 batches after passes
   3/5/7 so PE never idles long (HAM clock gate stays at 2.4 GHz).
 - PE warm-up: dummy matmuls during the initial DMA wait.
 - host-side layouts pre-scrambled so every DMA is 128 contiguous
   per-partition chunks.
"""

import math

import numpy as np
import ml_dtypes

import concourse.bass as bass
import concourse.mybir as mybir
import concourse.tile as tile
from concourse import bacc
from concourse.bass_utils import run_bass_kernel_spmd

L = 1024
D = 768
H = 12
FD = 16
HD = 64
NCORE = 8
NCH = 8
KB = 6
F32 = mybir.dt.float32
BF16 = mybir.dt.bfloat16

A_SCALE = 1.0 / math.sqrt(32.0)
A_BIAS = 1.0 / math.sqrt(2.0)

# feature flags gated on hardware probes
SINGLE_PASS_PROJ = True  # partition-shift DVE copy (probe T6: PASS)
WARM_MMS = 26

EPI_BATCHES = {3: (0, 4), 7: (4, 8)}

_compiled_nc = None
_last_in_maps = None
_last_res = None


def _bank_splits(lo, hi, bank=512):
    out = []
    a = lo
    while a < hi:
        b = min(hi, (a // bank + 1) * bank)
        out.append((a, b))
        a = b
    return out


def _use_act(j, blk, h):
    """Square-engine assignment: alternate ACT/DVE early; ACT-only in the
    late passes where DVE runs the epilogues."""
    if j >= 4:
        return True
    return (blk + h) % 2 == 0


def _build_nc():
    nc = bacc.Bacc("TRN2", target_bir_lowering=False, debug=False, num_devices=NCORE)

    hsp = nc.dram_tensor("hsp", [128, KB, L], BF16, kind="ExternalInput")
    wproj = nc.dram_tensor("wproj", [128, KB, 256], BF16, kind="ExternalInput")
    wod = nc.dram_tensor("wod", [128, D], BF16, kind="ExternalInput")
    cst = nc.dram_tensor("cst", [128, 320], BF16, kind="ExternalInput")
    seld = nc.dram_tensor("seld", [8, L], BF16, kind="ExternalInput")
    krow = nc.dram_tensor("krow", [1, 2048], BF16, kind="ExternalInput")
    outd = nc.dram_tensor("out", [128, NCH * D], BF16, kind="ExternalOutput")

    MUL = mybir.AluOpType.mult
    ADD = mybir.AluOpType.add
    SQF = mybir.ActivationFunctionType.Square

    with tile.TileContext(nc) as tc:
        with (
            tc.tile_pool(name="cst", bufs=1) as cp,
            tc.tile_pool(name="sqp", bufs=6) as sqp,
            tc.tile_pool(name="wrk", bufs=2) as wrk,
        ):
            hs_sb = [cp.tile([128, L], BF16, tag=f"hs{kb}", name=f"hs{kb}") for kb in range(KB)]
            wp_sb = cp.tile([128, KB, 256], BF16, tag="wp")
            wo_sb = cp.tile([128, D], BF16, tag="wo")
            cst_sb = cp.tile([128, 320], BF16, tag="cstt")
            sel_sb = cp.tile([8, L], BF16, tag="sel")
            kq_sb = cp.tile([64, 2048], BF16, tag="kq")
            vx_sb = cp.tile([128, NCH, 130], BF16, tag="vx")
            colsum_sb = cp.tile([8, 130], BF16, tag="colsum")
            osb = cp.tile([128, NCH, D], BF16, tag="osb")
            warm_sb = cp.tile([128, 128], BF16, tag="warm")
            ones64_sb = cp.tile([65, 64], F32, tag="ones64")
            wact_sb = cp.tile([128, 2], BF16, tag="wact")
            tri_sb = cst_sb[:, 0:128]
            htri_sb = cst_sb[:, 128:256]
            ones8_sb = cst_sb[:, 256:320]
            yT2 = cp.tile([128, L], BF16, tag="yT2")
            yT = [yT2[0:64, :], yT2[64:128, :]]

            # ---- input DMAs, priority-ordered ----
            nc.sync.dma_start(out=wp_sb, in_=wproj.ap())
            for kb in range(4):
                nc.sync.dma_start(out=hs_sb[kb], in_=hsp.ap()[:, kb, :])
            nc.sync.dma_start(out=cst_sb, in_=cst.ap())
            nc.sync.dma_start(out=sel_sb, in_=seld.ap())
            for kb in (4, 5):
                nc.sync.dma_start(out=hs_sb[kb], in_=hsp.ap()[:, kb, :])
            nc.sync.dma_start(out=wo_sb, in_=wod.ap())

            # ---- PE warm-up + ACT table preload during the DMA wait ----
            nc.gpsimd.memset(warm_sb, 0.0)
            nc.vector.memset(ones64_sb, 0.0)
            nc.vector.memset(ones64_sb[64:65, :], 1.0)
            nc.scalar.activation(out=wact_sb, in_=warm_sb[:, 0:2], func=SQF)
            with tc.tile_pool(name="pwarm", bufs=1, space="PSUM") as pw:
                pwt = pw.tile([128, 128], F32, tag="pw")
                for _ in range(WARM_MMS):
                    nc.tensor.matmul(pwt, warm_sb, warm_sb, start=True, stop=True)

            # ---- projections ----
            with (
                tc.tile_pool(name="pproj", bufs=1, space="PSUM") as pp,
                tc.tile_pool(name="pvp", bufs=3, space="PSUM") as pvp,
            ):
                if SINGLE_PASS_PROJ:
                    pkq = pp.tile([128, L], F32, tag="pkq")
                    for kb in range(KB):
                        for a, b in ((0, 512), (512, 1024)):
                            nc.tensor.matmul(
                                pkq[:, a:b], wp_sb[:, kb, 0:128], hs_sb[kb][:, a:b],
                                start=(kb == 0), stop=(kb == KB - 1),
                            )
                    nc.vector.tensor_copy(kq_sb[:, 0:1024], pkq[0:64, :])
                    # partition-shifting copy: q half (PSUM rows 64-127) -> base 0
                    nc.vector.tensor_copy(kq_sb[:, 1024:2048], pkq[64:128, :])
                else:
                    pk = pp.tile([64, L], F32, tag="pk")
                    pq = pp.tile([64, L], F32, tag="pq")
                    for kb in range(KB):
                        for a, b in ((0, 512), (512, 1024)):
                            nc.tensor.matmul(
                                pk[:, a:b], wp_sb[:, kb, 0:64], hs_sb[kb][:, a:b],
                                start=(kb == 0), stop=(kb == KB - 1),
                            )
                            nc.tensor.matmul(
                                pq[:, a:b], wp_sb[:, kb, 64:128], hs_sb[kb][:, a:b],
                                start=(kb == 0), stop=(kb == KB - 1),
                            )
                    nc.vector.tensor_copy(kq_sb[:, 0:1024], pk)
                    nc.vector.tensor_copy(kq_sb[:, 1024:2048], pq)

                # v projection: one full-bank PSUM tile per chunk so each
                # bank holds exactly one accumulation group (start=True
                # clears at bank granularity)
                for ch in range(NCH):
                    pv = pvp.tile([128, 512], F32, tag="pv", name=f"pv{ch}")
                    for kb in range(KB):
                        nc.tensor.matmul(
                            pv[:, 0:128],
                            hs_sb[kb][:, ch * 128 : (ch + 1) * 128],
                            wp_sb[:, kb, 128:256],
                            start=(kb == 0), stop=(kb == KB - 1),
                        )
                    nc.scalar.copy(out=vx_sb[:, ch, 0:64], in_=pv[:, 0:64])
                    nc.scalar.copy(out=vx_sb[:, ch, 65:129], in_=pv[:, 64:128])

                # constant rows (b on k side, 1 on q side) via tiny DMAs
                nc.sync.dma_start(out=kq_sb[0:1, :], in_=krow.ap())
                nc.sync.dma_start(out=kq_sb[32:33, :], in_=krow.ap())

                nc.gpsimd.memset(vx_sb[:, :, 64], 1.0)
                nc.gpsimd.memset(vx_sb[:, :, 129], 1.0)

            # per-chunk column sums of vx (inter-chunk +1/2 term);
            # h1 placed at partitions 32-39 for row-group concurrency
            with tc.tile_pool(name="pcsp", bufs=1, space="PSUM") as pcp:
                pcs = pcp.tile([8, 130], F32, tag="pcs")
                for ch in range(NCH):
                    nc.tensor.matmul(
                        pcs, ones8_sb[:, ch * 8 : (ch + 1) * 8], vx_sb[:, ch, :],
                        start=(ch == 0), stop=(ch == NCH - 1),
                    )
                nc.vector.tensor_copy(colsum_sb, pcs)

            # ================= attention =================
            with (
                tc.tile_pool(name="pnum", bufs=1, space="PSUM") as pn,
                tc.tile_pool(name="ppa", bufs=2, space="PSUM") as ppa,
                tc.tile_pool(name="ppo", bufs=1, space="PSUM") as ppo,
                tc.tile_pool(name="prbp", bufs=1, space="PSUM") as prbp,
            ):
                nums = [pn.tile([65, L], F32, tag=f"num{h}", name=f"num{h}") for h in range(2)]
                for j in range(NCH):
                    tlo = j * 128
                    w = L - tlo
                    nblk = (w + 511) // 512
                    sqs = []
                    for blk in range(nblk):
                        bw = min(512, w - blk * 512)
                        for h in range(2):
                            pa = ppa.tile([128, 512], F32, tag="pa", name=f"pa{j}_{blk}_{h}")[:, :bw]
                            nc.tensor.matmul(
                                pa,
                                kq_sb[32 * h : 32 * h + 32, tlo : tlo + 128],
                                kq_sb[
                                    32 * h : 32 * h + 32,
                                    1024 + tlo + blk * 512 : 1024 + tlo + blk * 512 + bw,
                                ],
                                start=True, stop=True,
                            )
                            sq = sqp.tile([128, 512], BF16, tag="sq", name=f"sq{j}_{blk}_{h}")[:, :bw]
                            if _use_act(j, blk, h):
                                nc.scalar.activation(out=sq, in_=pa, func=SQF)
                            else:
                                # DVE can't read two PSUM operands: bounce to
                                # SBUF (2x-accel copy), square on GpSimd
                                pb = sqp.tile([128, 512], BF16, tag="pb", name=f"pb{j}_{blk}_{h}")[:, :bw]
                                nc.vector.tensor_copy(pb, pa)
                                nc.gpsimd.tensor_mul(sq, pb, pb)
                            if blk == 0:
                                # causal mask; +1/2 intra term added via htri MM
                                nc.gpsimd.tensor_mul(sq[:, 0:128], sq[:, 0:128], tri_sb)
                            sqs.append((blk, h, sq, bw))
                    for blk, h, sq, bw in sqs:
                        lo = tlo + blk * 512
                        for a, b in _bank_splits(lo, lo + bw):
                            nc.tensor.matmul(
                                nums[h][:, a:b],
                                vx_sb[:, j, 65 * h : 65 * h + 65],
                                sq[:, a - lo : b - lo],
                                start=(j == 0), stop=False,
                            )
                    for h in range(2):
                        # intra-chunk +1/2 term: 0.5 * prefix-sums of V_j
                        nc.tensor.matmul(
                            nums[h][:, tlo : tlo + 128],
                            vx_sb[:, j, 65 * h : 65 * h + 65],
                            htri_sb,
                            start=False, stop=False,
                        )

                    if j in EPI_BATCHES:
                        c0, c1 = EPI_BATCHES[j]
                        # inter-chunk +1/2 term, closes each chunk's accumulation
                        for i in range(c0, c1):
                            for h in range(2):
                                nc.tensor.matmul(
                                    nums[h][:, i * 128 : (i + 1) * 128],
                                    colsum_sb[:, 65 * h : 65 * h + 65],
                                    sel_sb[:, i * 128 : (i + 1) * 128],
                                    start=False, stop=True,
                                )
                        lo, hi = c0 * 128, c1 * 128
                        ncols = hi - lo
                        for h in range(2):
                            rc = wrk.tile([65, 512], F32, tag="rc", name=f"rc{j}_{h}")[:, :ncols]
                            nc.vector.reciprocal_approx_fast(out=rc, in_=nums[h][:, lo:hi])
                            rb = wrk.tile([64, 512], F32, tag="rb", name=f"rb{j}_{h}")[:, :ncols]
                            for p0 in range(0, ncols, 256):
                                p1 = min(ncols, p0 + 256)
                                prb = prbp.tile([64, 256], F32, tag="prb", name=f"prb{j}_{h}_{p0}")[:, : p1 - p0]
                                nc.tensor.matmul(
                                    prb, ones64_sb[64:65, :], rc[64:65, p0:p1],
                                    start=True, stop=True,
                                )
                                nc.vector.tensor_copy(rb[:, p0:p1], prb)
                            # h1 write is partition-shifting (probe T9)
                            nc.vector.tensor_mul(yT[h][:, lo:hi], nums[h][0:64, lo:hi], rb)
                        for i in range(c0, c1):
                            for a, b in ((0, 512), (512, D)):
                                po = ppo.tile([128, 512], F32, tag="po", name=f"po{i}_{a}")[:, : b - a]
                                nc.tensor.matmul(
                                    po, yT2[:, i * 128 : (i + 1) * 128],
                                    wo_sb[:, a:b],
                                    start=True, stop=True,
                                )
                                if i % 2 == 0:
                                    nc.scalar.copy(out=osb[:, i, a:b], in_=po)
                                else:
                                    nc.vector.tensor_copy(osb[:, i, a:b], po)
                            nc.sync.dma_start(
                                out=outd.ap()[:, i * D : (i + 1) * D], in_=osb[:, i, :]
                            )

    nc.finalize()
    return nc


def _host_consts():
    s = np.arange(128)[:, None]
    t = np.arange(128)[None, :]
    tri = (s <= t).astype(np.float32)
    htri = 0.5 * tri
    ones8 = np.zeros((128, 64), dtype=np.float32)
    for ch in range(NCH):
        ones8[:, ch * 8 + ch] = 1.0
    sel = np.zeros((8, L), dtype=np.float32)
    for i in range(NCH):
        sel[:i, i * 128 : (i + 1) * 128] = 0.5
    return tri, htri, ones8, sel


def kernel(hidden_states, Wq, Wk, Wv, Wo):
    global _compiled_nc, _last_in_maps
    hs = np.asarray(hidden_states, dtype=np.float32)[0]  # [L, D]
    Wq = np.asarray(Wq, dtype=np.float32)
    Wk = np.asarray(Wk, dtype=np.float32)
    Wv = np.asarray(Wv, dtype=np.float32)
    Wo = np.asarray(Wo, dtype=np.float32)

    if _compiled_nc is None:
        _compiled_nc = _build_nc()
    nc = _compiled_nc

    bf = ml_dtypes.bfloat16
    # hs scrambled: hsp[p, kb, l] = hs[l, kb*128+p]
    hsp = np.ascontiguousarray(
        hs.T.reshape(KB, 128, L).transpose(1, 0, 2)
    ).astype(bf)

    tri, htri, ones8, sel = _host_consts()
    cstm = np.zeros((128, 320), dtype=np.float32)
    cstm[:, 0:128] = tri
    cstm[:, 128:256] = htri
    cstm[:, 256:320] = ones8
    cstm = cstm.astype(bf)
    sel = sel.astype(bf)
    krow_h = np.zeros((1, 2048), dtype=np.float32)
    krow_h[0, 0:1024] = A_SCALE * 0 + A_BIAS
    krow_h[0, 1024:2048] = 1.0
    krow_h = krow_h.astype(bf)

    in_maps = []
    for c in range(NCORE):
        heads = [2 * c, 2 * c + 1]
        wp = np.zeros((D, 256), dtype=np.float32)
        wo_c = np.zeros((128, D), dtype=np.float32)
        for hi, h in enumerate(heads):
            if h >= H:
                continue
            wp[:, 1 + 32 * hi : 1 + 32 * hi + FD] = A_SCALE * Wk[:, h * FD : (h + 1) * FD]
            wp[:, 65 + 32 * hi : 65 + 32 * hi + FD] = Wq[:, h * FD : (h + 1) * FD]
            wp[:, 128 + 64 * hi : 128 + 64 * hi + HD] = Wv[:, h * HD : (h + 1) * HD]
            wo_c[64 * hi : 64 * hi + HD, :] = Wo[h * HD : (h + 1) * HD, :]
        wp_s = np.ascontiguousarray(
            wp.reshape(KB, 128, 256).transpose(1, 0, 2)
        ).astype(bf)
        in_maps.append(
            {
                "hsp": hsp,
                "wproj": wp_s,
                "wod": wo_c.astype(bf),
                "cst": cstm,
                "seld": sel,
                "krow": krow_h,
            }
        )

    _last_in_maps = in_maps
    res = run_bass_kernel_spmd(nc, in_maps, list(range(NCORE)))
    global _last_res
    _last_res = res
    acc = np.zeros((128, NCH, D), dtype=np.float32)
    for c in range(NCORE):
        acc += np.asarray(res.results[c]["out"], dtype=np.float32).reshape(
            128, NCH, D
        )
    out = acc.transpose(1, 0, 2).reshape(L, D)
    return out.reshape(1, L, D)


# revision 21
# speedup vs baseline: 1.0071x; 1.0023x over previous
"""Based-style linear attention (Taylor feature map) on 8 Trainium2 cores.

Math: per head h (FDIM=16, HEAD_DIM=64):
    q,k = HS@Wq, HS@Wk (16 dims/head), v = HS@Wv (64 dims/head)
    phi(q).phi(k) = Square(S/sqrt(32) + 1/sqrt(2)) + 1/2,  S = q.k
    y_t = sum_{s<=t} sq'_st v_s / sum_{s<=t} sq'_st ;  out = concat_h(y) @ Wo

Sharding: head-parallel, 2 virtual heads per core (12 real + 4 zero),
no collectives; host sums the 8 partial outputs.

v2 design vs baseline:
 - single-pass packed q/k projection: stationary [128,128] produces k at
   PSUM rows 0-63 and q at rows 64-127 in one matmul stream; the q half
   lands at SBUF base 0 via a partition-shifting DVE copy.
 - affine fold: k scaled by 1/sqrt(32) on host, constant rows (b on the
   k side, 1 on the q side) via memset, so PSUM holds S' = aS + b and the
   Square needs no scale/bias.
 - squares split between ACT (plain Square) and DVE (tensor_mul);
   diagonal-block causal mask fused with the +1/2 intra term on GpSimd:
   (sq + 0.5) * tri in one scalar_tensor_tensor.
 - v projection accumulated kb-outer (overlaps the hs DMA stream) into
   one PSUM tile [128, 8, 128]; per-head score matmuls run on PE row
   groups 0/32 concurrently (data placed at partitions 0-31/32-63).
 - chunk epilogues (sel +1/2 term at row groups 0/32, reciprocal, den
   broadcast via GpSimd partition_broadcast, y = num/den, o-proj at row
   groups 0/64, output copy + DMA) pipelined in 3 batches after passes
   3/5/7 so PE never idles long (HAM clock gate stays at 2.4 GHz).
 - PE warm-up: dummy matmuls during the initial DMA wait.
 - host-side layouts pre-scrambled so every DMA is 128 contiguous
   per-partition chunks.
"""

import math

import numpy as np
import ml_dtypes

import concourse.bass as bass
import concourse.mybir as mybir
import concourse.tile as tile
from concourse import bacc
from concourse.bass_utils import run_bass_kernel_spmd

L = 1024
D = 768
H = 12
FD = 16
HD = 64
NCORE = 8
NCH = 8
KB = 6
F32 = mybir.dt.float32
BF16 = mybir.dt.bfloat16

A_SCALE = 1.0 / math.sqrt(32.0)
A_BIAS = 1.0 / math.sqrt(2.0)

# feature flags gated on hardware probes
SINGLE_PASS_PROJ = True  # partition-shift DVE copy (probe T6: PASS)
WARM_MMS = 8

EPI_BATCHES = {3: (0, 4), 5: (4, 6), 7: (6, 8)}

_compiled_nc = None
_last_in_maps = None
_last_res = None


def _bank_splits(lo, hi, bank=512):
    out = []
    a = lo
    while a < hi:
        b = min(hi, (a // bank + 1) * bank)
        out.append((a, b))
        a = b
    return out


def _use_act(j, blk, h):
    """Square-engine assignment: alternate ACT/DVE early; ACT-only in the
    late passes where DVE runs the epilogues."""
    if j >= 4:
        return True
    return (blk + h) % 2 == 0


def _build_nc():
    nc = bacc.Bacc("TRN2", target_bir_lowering=False, debug=False, num_devices=NCORE)

    hsp = nc.dram_tensor("hsp", [128, KB, L], BF16, kind="ExternalInput")
    wproj = nc.dram_tensor("wproj", [128, KB, 256], BF16, kind="ExternalInput")
    wod = nc.dram_tensor("wod", [128, D], BF16, kind="ExternalInput")
    cst = nc.dram_tensor("cst", [128, 448], BF16, kind="ExternalInput")
    seld = nc.dram_tensor("seld", [8, L], BF16, kind="ExternalInput")
    krow = nc.dram_tensor("krow", [1, 2048], BF16, kind="ExternalInput")
    outd = nc.dram_tensor("out", [128, NCH * D], BF16, kind="ExternalOutput")

    MUL = mybir.AluOpType.mult
    ADD = mybir.AluOpType.add
    SQF = mybir.ActivationFunctionType.Square

    with tile.TileContext(nc) as tc:
        with (
            tc.tile_pool(name="cst", bufs=1) as cp,
            tc.tile_pool(name="sqp", bufs=6) as sqp,
            tc.tile_pool(name="wrk", bufs=2) as wrk,
        ):
            hs_sb = [cp.tile([128, L], BF16, tag=f"hs{kb}", name=f"hs{kb}") for kb in range(KB)]
            wp_sb = cp.tile([128, KB, 256], BF16, tag="wp")
            wo_sb = cp.tile([128, D], BF16, tag="wo")
            cst_sb = cp.tile([128, 448], BF16, tag="cstt")
            sel_sb = cp.tile([8, L], BF16, tag="sel")
            kq_sb = cp.tile([64, 2048], BF16, tag="kq")
            vx_sb = cp.tile([128, NCH, 130], BF16, tag="vx")
            colsum_sb = cp.tile([8, 130], BF16, tag="colsum")
            osb = cp.tile([128, NCH, D], BF16, tag="osb")
            warm_sb = cp.tile([128, 512], BF16, tag="warm")
            ones64_sb = cp.tile([65, 64], F32, tag="ones64")
            wact_sb = cp.tile([128, 2], BF16, tag="wact")
            tri_sb = cst_sb[:, 0:128]
            htri_sb = cst_sb[:, 128:256]
            ones8_sb = cst_sb[:, 256:320]
            ident_sb = cst_sb[:, 320:448]
            yT2 = cp.tile([128, L], BF16, tag="yT2")
            yT = [yT2[0:64, :], yT2[64:128, :]]

            # ---- input DMAs, priority-ordered ----
            nc.sync.dma_start(out=wp_sb, in_=wproj.ap())
            for kb in range(4):
                nc.sync.dma_start(out=hs_sb[kb], in_=hsp.ap()[:, kb, :])
            nc.sync.dma_start(out=cst_sb, in_=cst.ap())
            nc.sync.dma_start(out=sel_sb, in_=seld.ap())
            for kb in (4, 5):
                nc.sync.dma_start(out=hs_sb[kb], in_=hsp.ap()[:, kb, :])
            nc.sync.dma_start(out=wo_sb, in_=wod.ap())

            # ---- PE warm-up + ACT table preload during the DMA wait ----
            nc.gpsimd.memset(warm_sb, 0.0)
            nc.vector.memset(ones64_sb, 0.0)
            nc.vector.memset(ones64_sb[64:65, :], 1.0)
            nc.scalar.activation(out=wact_sb, in_=warm_sb[:, 0:2], func=SQF)
            with tc.tile_pool(name="pwarm", bufs=1, space="PSUM") as pw:
                pwt = pw.tile([128, 512], F32, tag="pw")
                for _ in range(WARM_MMS):
                    nc.tensor.matmul(pwt, warm_sb[:, 0:128], warm_sb, start=True, stop=True)

            # ---- projections ----
            with (
                tc.tile_pool(name="pproj", bufs=1, space="PSUM") as pp,
                tc.tile_pool(name="pvp", bufs=3, space="PSUM") as pvp,
            ):
                if SINGLE_PASS_PROJ:
                    pkq = pp.tile([128, L], F32, tag="pkq")
                    for kb in range(KB):
                        for a, b in ((0, 512), (512, 1024)):
                            nc.tensor.matmul(
                                pkq[:, a:b], wp_sb[:, kb, 0:128], hs_sb[kb][:, a:b],
                                start=(kb == 0), stop=(kb == KB - 1),
                            )
                    nc.vector.tensor_copy(kq_sb[:, 0:1024], pkq[0:64, :])
                    # partition-shifting copy: q half (PSUM rows 64-127) -> base 0
                    nc.vector.tensor_copy(kq_sb[:, 1024:2048], pkq[64:128, :])
                else:
                    pk = pp.tile([64, L], F32, tag="pk")
                    pq = pp.tile([64, L], F32, tag="pq")
                    for kb in range(KB):
                        for a, b in ((0, 512), (512, 1024)):
                            nc.tensor.matmul(
                                pk[:, a:b], wp_sb[:, kb, 0:64], hs_sb[kb][:, a:b],
                                start=(kb == 0), stop=(kb == KB - 1),
                            )
                            nc.tensor.matmul(
                                pq[:, a:b], wp_sb[:, kb, 64:128], hs_sb[kb][:, a:b],
                                start=(kb == 0), stop=(kb == KB - 1),
                            )
                    nc.vector.tensor_copy(kq_sb[:, 0:1024], pk)
                    nc.vector.tensor_copy(kq_sb[:, 1024:2048], pq)

                # v projection, transposed: vT[c, pos] = Wv.T @ hs in 12
                # wide matmuls, then 8 PE transposes back to [pos, c]
                pvt = pp.tile([128, L], F32, tag="pvt")
                for kb in range(KB):
                    for a, b in ((0, 512), (512, 1024)):
                        nc.tensor.matmul(
                            pvt[:, a:b], wp_sb[:, kb, 128:256], hs_sb[kb][:, a:b],
                            start=(kb == 0), stop=(kb == KB - 1),
                        )
                vt_sb = cp.tile([128, L], BF16, tag="vt")
                nc.vector.tensor_copy(vt_sb[:, 0:512], pvt[:, 0:512])
                nc.vector.tensor_copy(vt_sb[:, 512:1024], pvt[:, 512:1024])
                for ch in range(NCH):
                    ptr = pvp.tile([128, 128], BF16, tag="ptr", name=f"ptr{ch}")
                    nc.tensor.transpose(
                        ptr, vt_sb[:, ch * 128 : (ch + 1) * 128], ident_sb
                    )
                    nc.scalar.copy(out=vx_sb[:, ch, 0:64], in_=ptr[:, 0:64])
                    nc.scalar.copy(out=vx_sb[:, ch, 65:129], in_=ptr[:, 64:128])

                # constant rows (b on k side, 1 on q side) via tiny DMAs
                nc.sync.dma_start(out=kq_sb[0:1, :], in_=krow.ap())
                nc.sync.dma_start(out=kq_sb[32:33, :], in_=krow.ap())

                nc.gpsimd.memset(vx_sb[:, :, 64], 1.0)
                nc.gpsimd.memset(vx_sb[:, :, 129], 1.0)

            # per-chunk column sums of vx (inter-chunk +1/2 term);
            # h1 placed at partitions 32-39 for row-group concurrency
            with tc.tile_pool(name="pcsp", bufs=1, space="PSUM") as pcp:
                pcs = pcp.tile([8, 130], F32, tag="pcs")
                for ch in range(NCH):
                    nc.tensor.matmul(
                        pcs, ones8_sb[:, ch * 8 : (ch + 1) * 8], vx_sb[:, ch, :],
                        start=(ch == 0), stop=(ch == NCH - 1),
                    )
                nc.vector.tensor_copy(colsum_sb, pcs)

            # ================= attention =================
            with (
                tc.tile_pool(name="pnum", bufs=1, space="PSUM") as pn,
                tc.tile_pool(name="ppa", bufs=2, space="PSUM") as ppa,
                tc.tile_pool(name="ppo", bufs=1, space="PSUM") as ppo,
                tc.tile_pool(name="prbp", bufs=1, space="PSUM") as prbp,
            ):
                nums = [pn.tile([65, L], F32, tag=f"num{h}", name=f"num{h}") for h in range(2)]
                for j in range(NCH):
                    tlo = j * 128
                    w = L - tlo
                    nblk = (w + 511) // 512
                    sqs = []
                    for blk in range(nblk):
                        bw = min(512, w - blk * 512)
                        for h in range(2):
                            pa = ppa.tile([128, 512], F32, tag="pa", name=f"pa{j}_{blk}_{h}")[:, :bw]
                            nc.tensor.matmul(
                                pa,
                                kq_sb[32 * h : 32 * h + 32, tlo : tlo + 128],
                                kq_sb[
                                    32 * h : 32 * h + 32,
                                    1024 + tlo + blk * 512 : 1024 + tlo + blk * 512 + bw,
                                ],
                                start=True, stop=True,
                            )
                            sq = sqp.tile([128, 512], BF16, tag="sq", name=f"sq{j}_{blk}_{h}")[:, :bw]
                            if _use_act(j, blk, h):
                                nc.scalar.activation(out=sq, in_=pa, func=SQF)
                            else:
                                # DVE can't read two PSUM operands: bounce to
                                # SBUF (2x-accel copy), square on GpSimd
                                pb = sqp.tile([128, 512], BF16, tag="pb", name=f"pb{j}_{blk}_{h}")[:, :bw]
                                nc.vector.tensor_copy(pb, pa)
                                nc.gpsimd.tensor_mul(sq, pb, pb)
                            if blk == 0:
                                # causal mask; +1/2 intra term added via htri MM
                                nc.gpsimd.tensor_mul(sq[:, 0:128], sq[:, 0:128], tri_sb)
                            sqs.append((blk, h, sq, bw))
                    for blk, h, sq, bw in sqs:
                        lo = tlo + blk * 512
                        for a, b in _bank_splits(lo, lo + bw):
                            nc.tensor.matmul(
                                nums[h][:, a:b],
                                vx_sb[:, j, 65 * h : 65 * h + 65],
                                sq[:, a - lo : b - lo],
                                start=(j == 0), stop=False,
                            )
                    for h in range(2):
                        # intra-chunk +1/2 term: 0.5 * prefix-sums of V_j
                        nc.tensor.matmul(
                            nums[h][:, tlo : tlo + 128],
                            vx_sb[:, j, 65 * h : 65 * h + 65],
                            htri_sb,
                            start=False, stop=False,
                        )

                    if j in EPI_BATCHES:
                        c0, c1 = EPI_BATCHES[j]
                        # inter-chunk +1/2 term, closes the batch's accumulation
                        for h in range(2):
                            nc.tensor.matmul(
                                nums[h][:, c0 * 128 : c1 * 128],
                                colsum_sb[:, 65 * h : 65 * h + 65],
                                sel_sb[:, c0 * 128 : c1 * 128],
                                start=False, stop=True,
                            )
                        lo, hi = c0 * 128, c1 * 128
                        ncols = hi - lo
                        for h in range(2):
                            rc = wrk.tile([65, 512], F32, tag="rc", name=f"rc{j}_{h}")[:, :ncols]
                            nc.vector.reciprocal_approx_fast(out=rc, in_=nums[h][:, lo:hi])
                            rb = wrk.tile([64, 512], F32, tag="rb", name=f"rb{j}_{h}")[:, :ncols]
                            prb = prbp.tile([64, 512], F32, tag="prb", name=f"prb{j}_{h}")[:, :ncols]
                            nc.tensor.matmul(
                                prb, ones64_sb[64:65, :], rc[64:65, :],
                                start=True, stop=True,
                            )
                            nc.vector.tensor_copy(rb, prb)
                            # h1 write is partition-shifting (probe T9)
                            nc.vector.tensor_mul(yT[h][:, lo:hi], nums[h][0:64, lo:hi], rb)
                        for i in range(c0, c1):
                            for a, b in ((0, 512), (512, D)):
                                po = ppo.tile([128, 512], F32, tag="po", name=f"po{i}_{a}")[:, : b - a]
                                nc.tensor.matmul(
                                    po, yT2[:, i * 128 : (i + 1) * 128],
                                    wo_sb[:, a:b],
                                    start=True, stop=True,
                                )
                                if i % 2 == 0:
                                    nc.scalar.copy(out=osb[:, i, a:b], in_=po)
                                else:
                                    nc.vector.tensor_copy(osb[:, i, a:b], po)
                            nc.sync.dma_start(
                                out=outd.ap()[:, i * D : (i + 1) * D], in_=osb[:, i, :]
                            )

    nc.finalize()
    return nc


def _host_consts():
    s = np.arange(128)[:, None]
    t = np.arange(128)[None, :]
    tri = (s <= t).astype(np.float32)
    htri = 0.5 * tri
    ones8 = np.zeros((128, 64), dtype=np.float32)
    for ch in range(NCH):
        ones8[:, ch * 8 + ch] = 1.0
    sel = np.zeros((8, L), dtype=np.float32)
    for i in range(NCH):
        sel[:i, i * 128 : (i + 1) * 128] = 0.5
    return tri, htri, ones8, sel


def kernel(hidden_states, Wq, Wk, Wv, Wo):
    global _compiled_nc, _last_in_maps
    hs = np.asarray(hidden_states, dtype=np.float32)[0]  # [L, D]
    Wq = np.asarray(Wq, dtype=np.float32)
    Wk = np.asarray(Wk, dtype=np.float32)
    Wv = np.asarray(Wv, dtype=np.float32)
    Wo = np.asarray(Wo, dtype=np.float32)

    if _compiled_nc is None:
        _compiled_nc = _build_nc()
    nc = _compiled_nc

    bf = ml_dtypes.bfloat16
    # hs scrambled: hsp[p, kb, l] = hs[l, kb*128+p]
    hsp = np.ascontiguousarray(
        hs.T.reshape(KB, 128, L).transpose(1, 0, 2)
    ).astype(bf)

    tri, htri, ones8, sel = _host_consts()
    cstm = np.zeros((128, 448), dtype=np.float32)
    cstm[:, 0:128] = tri
    cstm[:, 128:256] = htri
    cstm[:, 256:320] = ones8
    cstm[:, 320:448] = np.eye(128, dtype=np.float32)
    cstm = cstm.astype(bf)
    sel = sel.astype(bf)
    krow_h = np.zeros((1, 2048), dtype=np.float32)
    krow_h[0, 0:1024] = A_SCALE * 0 + A_BIAS
    krow_h[0, 1024:2048] = 1.0
    krow_h = krow_h.astype(bf)

    in_maps = []
    for c in range(NCORE):
        heads = [2 * c, 2 * c + 1]
        wp = np.zeros((D, 256), dtype=np.float32)
        wo_c = np.zeros((128, D), dtype=np.float32)
        for hi, h in enumerate(heads):
            if h >= H:
                continue
            wp[:, 1 + 32 * hi : 1 + 32 * hi + FD] = A_SCALE * Wk[:, h * FD : (h + 1) * FD]
            wp[:, 65 + 32 * hi : 65 + 32 * hi + FD] = Wq[:, h * FD : (h + 1) * FD]
            wp[:, 128 + 64 * hi : 128 + 64 * hi + HD] = Wv[:, h * HD : (h + 1) * HD]
            wo_c[64 * hi : 64 * hi + HD, :] = Wo[h * HD : (h + 1) * HD, :]
        wp_s = np.ascontiguousarray(
            wp.reshape(KB, 128, 256).transpose(1, 0, 2)
        ).astype(bf)
        in_maps.append(
            {
                "hsp": hsp,
                "wproj": wp_s,
                "wod": wo_c.astype(bf),
                "cst": cstm,
                "seld": sel,
                "krow": krow_h,
            }
        )

    _last_in_maps = in_maps
    res = run_bass_kernel_spmd(nc, in_maps, list(range(NCORE)))
    global _last_res
    _last_res = res
    acc = np.zeros((128, NCH, D), dtype=np.float32)
    for c in range(NCORE):
        acc += np.asarray(res.results[c]["out"], dtype=np.float32).reshape(
            128, NCH, D
        )
    out = acc.transpose(1, 0, 2).reshape(L, D)
    return out.reshape(1, L, D)
